# revision 1
# baseline (speedup 1.0000x reference)
"""Trainium2 Bass kernel for BertTempRel-style span-mean + MLP + softmax.

Reference computation (per batch row b of x[B, S, D]):
    e1 = mean(x[b, lo1:hi1, :]),  e2 = mean(x[b, lo2:hi2, :]),  cls = x[b, 0, :]
    (hi = max(hi, lo+1) empty-span guard)
    feat = concat([e1, e2, cls])            # [3D]
    out = softmax(relu(relu(feat@W1+b1)@W2+b2)@W3+b3)

Strategy: pure data-parallel over 8 NeuronCores (128 batch rows each).
Per core, x is streamed once (memory-roofline bound). The span means are
computed on the TensorEngine: for each b, the tiny 0/1 span masks (plus a
one-hot row for CLS) form the *stationary* operand [128s, 3] and the x
tile [128s, D] is the *moving* operand, accumulating [3, D] span sums in
PSUM over the 4 s-chunks (float32r mode: 1 cycle/row at N>=256). PSUM is
evacuated with a fused 1/count scale, transposed back to [d, b] layout via
tiny PE transposes, and the whole 128-row MLP runs as a handful of
matmuls at the end.
"""

import sys

if "/opt/trn_rl_repo" not in sys.path:
    sys.path.insert(0, "/opt/trn_rl_repo")

import numpy as np

from concourse import bacc, bass, mybir, tile
from concourse.bass_utils import run_bass_kernel_spmd
from concourse.masks import make_identity

F32 = mybir.dt.float32
F32R = mybir.dt.float32r
I32 = mybir.dt.int32
OP = mybir.AluOpType
AF = mybir.ActivationFunctionType

N_CORES = 8
B_FULL, S, D = 1024, 512, 768
H1, H2, H3 = 256, 64, 4
BPC = B_FULL // N_CORES  # batch rows per core (128)
BPD = 2                  # batch rows loaded per DMA


def build_program(bpc=BPC, s=S, d=D, h1=H1, h2=H2, h3=H3, bpd=BPD, rep=1,
                  xbufs=3, skip=False):
    # skip=True predicates each 64-row slab DMA on its span masks being
    # nonzero (~24% fewer bytes on this data; compute unchanged since zero
    # masks nullify stale slabs). The packed-word form below (one value_load
    # per b, per-slab conds via (w >> k) & 1) clears both earlier compile
    # blockers (register exhaustion from hoisted per-slab loads; missing DGE
    # sync info under tile_critical) and passes walrus, but the resulting
    # NEFF fails at device execution through this axon deployment with an
    # opaque INTERNAL error. Left disabled; revisit where NRT diagnostics
    # are available.
    """Emit the per-core Bass/Tile program. All 8 cores run it SPMD."""
    sc = s // 128          # s-chunks
    dh = d // 2            # moving free-dim per span matmul
    nd = d // 128          # d-chunks of 128
    nf = 3 * d // 128      # feature chunks of 128
    nh1 = h1 // 128

    nc = bacc.Bacc("TRN2", target_bir_lowering=False, debug=False,
                   num_devices=N_CORES)

    x_d = nc.dram_tensor("x", [bpc, s, d], F32R, kind="ExternalInput")
    e1_d = nc.dram_tensor("e1", [bpc, 2], I32, kind="ExternalInput")
    e2_d = nc.dram_tensor("e2", [bpc, 2], I32, kind="ExternalInput")
    w1_d = nc.dram_tensor("W1", [nf, 128, h1], F32, kind="ExternalInput")
    b1_d = nc.dram_tensor("b1", [1, h1], F32, kind="ExternalInput")
    w2_d = nc.dram_tensor("W2", [nh1, 128, h2], F32, kind="ExternalInput")
    b2_d = nc.dram_tensor("b2", [1, h2], F32, kind="ExternalInput")
    w3_d = nc.dram_tensor("W3", [h2, h3], F32, kind="ExternalInput")
    b3_d = nc.dram_tensor("b3", [1, h3], F32, kind="ExternalInput")
    out_d = nc.dram_tensor("out", [bpc, h3], F32, kind="ExternalOutput")

    with tile.TileContext(nc) as tc:
        with tc.tile_pool(name="const", bufs=1) as const:
            ident = const.tile([128, 128], F32)
            make_identity(nc, ident[:])

            w1 = const.tile([128, nf, h1], F32)
            nc.sync.dma_start(w1[:], w1_d.ap().rearrange("p k h -> k p h"))
            w2 = const.tile([128, nh1, h2], F32)
            nc.sync.dma_start(w2[:], w2_d.ap().rearrange("p k h -> k p h"))
            w3 = const.tile([h2, h3], F32)
            nc.sync.dma_start(w3[:], w3_d.ap()[:])
            b1r = const.tile([1, h1], F32)
            nc.sync.dma_start(b1r[:], b1_d.ap()[:])
            b2r = const.tile([1, h2], F32)
            nc.sync.dma_start(b2r[:], b2_d.ap()[:])
            b3r = const.tile([1, h3], F32)
            nc.sync.dma_start(b3r[:], b3_d.ap()[:])
            ones = const.tile([1, 128], F32)
            nc.vector.memset(ones[:], 1.0)

            for _rep in range(rep):
                # ---- span bounds, counts, reciprocal counts ([b, *] layout) ----
                sp_i = const.tile([bpc, 4], I32)
                nc.sync.dma_start(sp_i[:, 0:2], e1_d.ap()[:])
                nc.sync.dma_start(sp_i[:, 2:4], e2_d.ap()[:])
                sp_f = const.tile([bpc, 4], F32)
                nc.vector.tensor_copy(sp_f[:], sp_i[:])

                bounds = const.tile([bpc, 4], F32)  # lo1, hi1, lo2, hi2 (guarded)
                rp = const.tile([bpc, 3], F32)      # 1/cnt1, 1/cnt2, 1.0
                cnt = const.tile([bpc, 2], F32)
                for j in range(2):
                    lo = sp_f[:, 2 * j:2 * j + 1]
                    hi_raw = sp_f[:, 2 * j + 1:2 * j + 2]
                    lo_out = bounds[:, 2 * j:2 * j + 1]
                    hi_out = bounds[:, 2 * j + 1:2 * j + 2]
                    nc.vector.tensor_copy(lo_out, lo)
                    # hi = max(hi_raw, lo+1); cnt = hi - lo; rp = 1/cnt
                    nc.vector.tensor_scalar(hi_out, lo, 1.0, None, OP.add)
                    nc.vector.tensor_tensor(hi_out, hi_raw, hi_out, OP.max)
                    nc.vector.tensor_tensor(cnt[:, j:j + 1], hi_out, lo_out,
                                            OP.subtract)
                    nc.vector.reciprocal(rp[:, j:j + 1], cnt[:, j:j + 1])
                nc.vector.memset(rp[:, 2:3], 1.0)

                # ---- masks in [b, s] layout ----
                iota_i = const.tile([bpc, s], I32)
                nc.gpsimd.iota(iota_i[:], pattern=[[1, s]], base=0,
                               channel_multiplier=0)
                iota_f = const.tile([bpc, s], F32)
                nc.vector.tensor_copy(iota_f[:], iota_i[:])

                masks = const.tile([bpc, 3, s], F32)
                ge = const.tile([bpc, s], F32)
                for j in range(2):
                    lo = bounds[:, 2 * j:2 * j + 1]
                    hi = bounds[:, 2 * j + 1:2 * j + 2]
                    nc.vector.tensor_scalar(ge[:], iota_f[:], lo, None, OP.is_ge)
                    nc.vector.scalar_tensor_tensor(masks[:, j, :], iota_f[:], hi,
                                                   ge[:], OP.is_lt, OP.mult)
                # CLS one-hot "mask": 1.0 at s == 0
                nc.vector.tensor_scalar(masks[:, 2, :], iota_f[:], 0.0, None,
                                        OP.is_equal)

                # ---- transpose masks/scales to [s, b] / [3, b] layouts ----
                # mt[s_p, c, b, m]: stationary operand source; m: e1, e2, cls.
                mt = const.tile([128, sc, bpc, 3], F32R)
                scl = const.tile([3, bpc], F32)
                with tc.tile_pool(name="p0psum", bufs=2, space="PSUM") as p0p:
                    for c in range(sc):
                        for j in range(3):
                            tp = p0p.tile([128, bpc], F32, tag="tp")
                            nc.tensor.transpose(tp[:], masks[:, j, bass.ts(c, 128)],
                                                ident[0:bpc, 0:bpc])
                            nc.vector.tensor_copy(mt[:, c, :, j], tp[:])
                    tps = p0p.tile([3, bpc], F32, tag="tps")
                    nc.tensor.transpose(tps[:], rp[:], ident[0:bpc, 0:bpc])
                    nc.vector.tensor_copy(scl[:], tps[:])

                # ---- per-(b, 64-row slab) load predicates ----
                # A slab whose mask columns are all zero contributes nothing
                # (zero mask x stale data = 0), so its DMA can be skipped.
                nsl = s // 64
                if skip:
                    st_i = const.tile([bpc, nsl], I32)
                    nc.gpsimd.iota(st_i[:], pattern=[[64, nsl]], base=0,
                                   channel_multiplier=0)
                    st_f = const.tile([bpc, nsl], F32)
                    nc.vector.tensor_copy(st_f[:], st_i[:])
                    en_f = const.tile([bpc, nsl], F32)
                    nc.vector.tensor_scalar(en_f[:], st_f[:], 64.0, None, OP.add)
                    nf1 = const.tile([bpc, nsl], F32)
                    nf2 = const.tile([bpc, nsl], F32)
                    for j, nf_t in ((0, nf1), (1, nf2)):
                        lo = bounds[:, 2 * j:2 * j + 1]
                        hi = bounds[:, 2 * j + 1:2 * j + 2]
                        # span intersects slab k iff end_k > lo and start_k < hi
                        nc.vector.tensor_scalar(nf_t[:], en_f[:], lo, None,
                                                OP.is_gt)
                        nc.vector.scalar_tensor_tensor(nf_t[:], st_f[:], hi,
                                                       nf_t[:], OP.is_lt, OP.mult)
                    need_f = const.tile([bpc, nsl], F32)
                    nc.vector.tensor_tensor(need_f[:], nf1[:], nf2[:], OP.max)
                    nc.vector.memset(need_f[:, 0:1], 1.0)  # CLS slab
                    # pack the nsl flags into one int word per b so the DMA
                    # loop needs a single register load per b
                    pw = const.tile([bpc, nsl], F32)
                    for k in range(nsl):
                        nc.vector.memset(pw[:, k:k + 1], float(1 << k))
                    nc.vector.tensor_tensor(pw[:], pw[:], need_f[:], OP.mult)
                    word_f = const.tile([bpc, 1], F32)
                    nc.vector.tensor_reduce(word_f[:], pw[:],
                                            mybir.AxisListType.X, OP.add)
                    word_i = const.tile([bpc, 1], I32)
                    nc.vector.tensor_copy(word_i[:], word_f[:])

                # packT[d_p, dc, b, m]: transposed scaled span sums / cls.
                packT = const.tile([128, nd, bpc, 3], F32)

                # ---- main loop: stream x, accumulate span sums on PE ----
                with tc.tile_pool(name="xp", bufs=xbufs) as xp, \
                     tc.tile_pool(name="stg", bufs=4) as stg, \
                     tc.tile_pool(name="sps0", bufs=2, space="PSUM") as sps0, \
                     tc.tile_pool(name="sps1", bufs=2, space="PSUM") as sps1, \
                     tc.tile_pool(name="ptp", bufs=2, space="PSUM") as ptp:
                    for i in range(bpc // bpd):
                        xb = xp.tile([128, bpd, sc, d], F32R, tag="xb")
                        if not skip:
                            dma_eng = nc.sync if i % 2 == 0 else nc.scalar
                            dma_eng.dma_start(
                                xb[:],
                                x_d.ap()[bpd * i:bpd * (i + 1)].rearrange(
                                    "b (c p) d -> p b c d", p=128))
                        else:
                            for j in range(bpd):
                                b = bpd * i + j
                                eng = nc.sync if b % 2 == 0 else nc.scalar
                                w = None
                                if i >= xbufs:
                                    w = eng.value_load(word_i[b:b + 1, 0:1],
                                                       min_val=0, max_val=255)
                                for k in range(nsl):
                                    c, ph = k // 2, 64 * (k % 2)
                                    dst = xb[ph:ph + 64, j, c, :]
                                    src = x_d.ap()[b, 64 * k:64 * (k + 1), :]
                                    if w is None or k == 0:
                                        # first tiles load fully so every slot
                                        # holds finite data before any skip
                                        eng.dma_start(dst, src)
                                    else:
                                        eng.dma_start(dst, src,
                                                      cond=(w >> k) & 1)
                        for j in range(bpd):
                            b = bpd * i + j
                            ps0 = sps0.tile([3, dh], F32, tag="ps0")
                            ps1 = sps1.tile([3, dh], F32, tag="ps1")
                            for c in range(sc):
                                lhsT = mt[:, c, b, :]
                                nc.tensor.matmul(ps0[:], lhsT,
                                                 xb[:, j, c, 0:dh],
                                                 start=(c == 0), stop=(c == sc - 1))
                                nc.tensor.matmul(ps1[:], lhsT,
                                                 xb[:, j, c, dh:d],
                                                 start=(c == 0), stop=(c == sc - 1))
                            # evacuate + scale by 1/cnt (split across DVE/ACT)
                            sg = stg.tile([3, d], F32, tag="sg")
                            nc.vector.tensor_scalar(sg[:, 0:dh], ps0[:],
                                                    scl[:, b:b + 1], None, OP.mult)
                            nc.scalar.mul(sg[:, dh:d], ps1[:], scl[:, b:b + 1])
                            # transpose [3, d] -> nd x [128, 3] columns of packT
                            for dc in range(nd):
                                pt = ptp.tile([128, 3], F32, tag="pt")
                                nc.tensor.transpose(pt[:], sg[:, bass.ts(dc, 128)],
                                                    ident[0:3, 0:3])
                                if dc % 2 == 0:
                                    nc.vector.tensor_copy(packT[:, dc, b, :], pt[:])
                                else:
                                    nc.scalar.copy(packT[:, dc, b, :], pt[:])

                # ---- de-interleave features: featT[f_p, p, b] ----
                featT = const.tile([128, nf, bpc], F32)
                for m in range(3):
                    for dc in range(nd):
                        nc.vector.tensor_copy(featT[:, m * nd + dc, :],
                                              packT[:, dc, :, m])

                # ---- MLP + softmax over all bpc rows at once ----
                h1s = const.tile([bpc, h1], F32)
                h1T = const.tile([128, nh1, bpc], F32)
                h2s = const.tile([bpc, h2], F32)
                h2T = const.tile([h2, bpc], F32)
                probs = const.tile([bpc, h3], F32)
                mx = const.tile([bpc, 1], F32)
                ex = const.tile([bpc, h3], F32)
                sm = const.tile([bpc, 1], F32)
                rc = const.tile([bpc, 1], F32)

                with tc.tile_pool(name="mlpp", bufs=1, space="PSUM") as mp:
                    h1p = mp.tile([bpc, h1], F32, tag="h1p")
                    for p in range(nf):
                        nc.tensor.matmul(h1p[:], featT[:, p, :], w1[:, p, :],
                                         start=(p == 0), stop=False)
                    nc.tensor.matmul(h1p[:], ones[0:1, 0:bpc], b1r[:],
                                     start=False, stop=True)
                    nc.scalar.activation(h1s[:], h1p[:], AF.Relu)

                    for k in range(nh1):
                        tp1 = mp.tile([128, bpc], F32, tag="tp1")
                        nc.tensor.transpose(tp1[:], h1s[:, bass.ts(k, 128)],
                                            ident[0:bpc, 0:bpc])
                        nc.vector.tensor_copy(h1T[:, k, :], tp1[:])

                    h2p = mp.tile([bpc, h2], F32, tag="h2p")
                    for k in range(nh1):
                        nc.tensor.matmul(h2p[:], h1T[:, k, :], w2[:, k, :],
                                         start=(k == 0), stop=False)
                    nc.tensor.matmul(h2p[:], ones[0:1, 0:bpc], b2r[:],
                                     start=False, stop=True)
                    nc.scalar.activation(h2s[:], h2p[:], AF.Relu)

                    tp2 = mp.tile([h2, bpc], F32, tag="tp2")
                    nc.tensor.transpose(tp2[:], h2s[:], ident[0:bpc, 0:bpc])
                    nc.vector.tensor_copy(h2T[:], tp2[:])

                    h3p = mp.tile([bpc, h3], F32, tag="h3p")
                    nc.tensor.matmul(h3p[:], h2T[:], w3[:], start=True, stop=False)
                    nc.tensor.matmul(h3p[:], ones[0:1, 0:bpc], b3r[:],
                                     start=False, stop=True)

                    # softmax along the 4 logits
                    nc.vector.tensor_reduce(mx[:], h3p[:], mybir.AxisListType.X,
                                            OP.max, negate=True)
                    nc.scalar.activation(ex[:], h3p[:], AF.Exp, bias=mx[:],
                                         scale=1.0)
                    nc.vector.tensor_reduce(sm[:], ex[:], mybir.AxisListType.X,
                                            OP.add)
                    nc.vector.reciprocal(rc[:], sm[:])
                    nc.vector.tensor_scalar(probs[:], ex[:], rc[:], None, OP.mult)

                nc.sync.dma_start(out_d.ap()[:], probs[:])

    nc.compile()
    return nc


_NC_CACHE = {}


def _get_program():
    if "nc" not in _NC_CACHE:
        _NC_CACHE["nc"] = build_program()
    return _NC_CACHE["nc"]


def make_in_maps(inputs):
    x = np.ascontiguousarray(np.asarray(inputs["x"], dtype=np.float32))
    e1 = np.ascontiguousarray(np.asarray(inputs["e1_span"], dtype=np.int32))
    e2 = np.ascontiguousarray(np.asarray(inputs["e2_span"], dtype=np.int32))
    w1 = np.ascontiguousarray(
        np.asarray(inputs["W1"], dtype=np.float32).reshape(3 * D // 128, 128, H1))
    b1 = np.asarray(inputs["b1"], dtype=np.float32).reshape(1, H1)
    w2 = np.ascontiguousarray(
        np.asarray(inputs["W2"], dtype=np.float32).reshape(H1 // 128, 128, H2))
    b2 = np.asarray(inputs["b2"], dtype=np.float32).reshape(1, H2)
    w3 = np.ascontiguousarray(np.asarray(inputs["W3"], dtype=np.float32))
    b3 = np.asarray(inputs["b3"], dtype=np.float32).reshape(1, H3)

    in_maps = []
    for c in range(N_CORES):
        sl = slice(c * BPC, (c + 1) * BPC)
        in_maps.append({
            "x": np.ascontiguousarray(x[sl]),
            "e1": np.ascontiguousarray(e1[sl]),
            "e2": np.ascontiguousarray(e2[sl]),
            "W1": w1, "b1": b1, "W2": w2, "b2": b2, "W3": w3, "b3": b3,
        })
    return in_maps


def kernel(**inputs) -> np.ndarray:
    nc = _get_program()
    res = run_bass_kernel_spmd(nc, make_in_maps(inputs),
                               core_ids=list(range(N_CORES)))
    return np.concatenate([res.results[c]["out"] for c in range(N_CORES)],
                          axis=0)



# revision 5
# speedup vs baseline: 1.7680x; 1.7680x over previous
"""Trainium2 Bass kernel for BertTempRel-style span-mean + MLP + softmax.

Reference computation (per batch row b of x[B, S, D]):
    e1 = mean(x[b, lo1:hi1, :]),  e2 = mean(x[b, lo2:hi2, :]),  cls = x[b, 0, :]
    (hi = max(hi, lo+1) empty-span guard)
    feat = concat([e1, e2, cls])            # [3D]
    out = softmax(relu(relu(feat@W1+b1)@W2+b2)@W3+b3)

Strategy: pure data-parallel over 8 NeuronCores. Only ~54% of x's rows are
inside the two spans (plus CLS), so instead of streaming all of x, each
core issues one SWDGE dma_gather per PAIR of batch rows that reads exactly
the union-of-spans rows into packed SBUF slots. The host derives the row
lists from the tiny span tensors, sorts the 1024 batch rows by union size
and deals them round-robin across cores (so the 8 rows sharing a loop
position have near-equal counts), then pairs head and tail positions so
every gather moves a near-constant ~560 rows. The per-position counts are
baked into the program as immediates (SPMD-safe: register-valued DMA
operands fail on this deployment); shorter rows pad with dummy row-0
reads. The 0/1 span masks (3 per batch row, 6 per pair) are precomputed
host-side directly in the transposed [slot, mask] layout the TensorEngine
needs, so the device does no mask computation at all: per pair it runs
cq ~= 5 accumulating matmuls (stationary masks x moving gathered rows),
scales the [6, D] PSUM result by host-provided 1/count factors, transposes
batches of 12 rows back to feature-major layout, and finishes with the
128-row MLP + softmax as a handful of matmuls.
"""

import sys

if "/opt/trn_rl_repo" not in sys.path:
    sys.path.insert(0, "/opt/trn_rl_repo")

import numpy as np

from concourse import bacc, bass, mybir, tile
from concourse.bass_utils import run_bass_kernel_spmd
from concourse.masks import make_identity

F32 = mybir.dt.float32
F32R = mybir.dt.float32r
I16 = mybir.dt.int16
I32 = mybir.dt.int32
OP = mybir.AluOpType
AF = mybir.ActivationFunctionType

N_CORES = 8
B_FULL, S, D = 1024, 512, 768
H1, H2, H3 = 256, 64, 4
BPC = B_FULL // N_CORES  # batch rows per core (128)
NP = BPC // 2            # gather pairs per core (64)


def derive_layout(e1_span, e2_span):
    """Host-side layout: row lists, core/pair assignment, masks, scales.

    Returns dict with
      rows[c]    = [BPC] global batch row for each storage slot of core c
      pcnt       = [NP, 2] baked per-sub-row counts (shared by all cores)
      cq         = [NP] slot chunks per pair gather
      gidx[c]    = [128, NP, 2*S//16] int16 wrapped gather index lists
      mt[c]      = [128, cqmax, NP, 6] float32 transposed span masks
      scl[c]     = [6, NP] float32 1/count scales (e1, e2, cls=1) x 2 rows
    """
    e1 = np.asarray(e1_span, dtype=np.int64)
    e2 = np.asarray(e2_span, dtype=np.int64)
    pos = np.arange(S)[None, :]
    lo1, hi1 = e1[:, 0:1], np.maximum(e1[:, 1:2], e1[:, 0:1] + 1)
    lo2, hi2 = e2[:, 0:1], np.maximum(e2[:, 1:2], e2[:, 0:1] + 1)
    m1 = (pos >= lo1) & (pos < hi1)
    m2 = (pos >= lo2) & (pos < hi2)
    m = m1 | m2
    m[:, 0] = True  # CLS row
    n = m.sum(axis=1)

    order = np.argsort(-n, kind="stable")
    perm = order.reshape(BPC, N_CORES)          # position i, core c
    counts = n[perm[:, 0]].astype(np.int64)     # descending => max of group

    # head-tail pairing: pair p serves positions (p, BPC-1-p)
    pcnt = np.stack([counts[:NP], counts[BPC - 1 - np.arange(NP)]], axis=1)
    cq = (pcnt.sum(axis=1) + 127) // 128
    cqmax = int(cq.max())

    rows, gidx, mt, scl = [], [], [], []
    for c in range(N_CORES):
        rows_c = np.empty(BPC, dtype=np.int64)
        rows_c[0::2] = perm[:NP, c]
        rows_c[1::2] = perm[BPC - 1 - np.arange(NP), c]
        rows.append(rows_c)

        idx_c = np.zeros((NP, 2 * S), dtype=np.int16)
        mt_c = np.zeros((128, cqmax, NP, 6), dtype=np.float32)
        scl_c = np.empty((6, NP), dtype=np.float32)
        for p in range(NP):
            off = 0
            for jj in range(2):
                gb = rows_c[2 * p + jj]
                u = np.flatnonzero(m[gb])
                k = int(pcnt[p, jj])
                # slot s of this pair holds source row idx within the
                # pair's 2S-row window: sub-row jj lives at jj*S + row
                idx_c[p, off:off + len(u)] = jj * S + u
                # dummy padding reads row 0 of sub-row jj (mask 0)
                idx_c[p, off + len(u):off + k] = jj * S
                slots = off + np.arange(len(u))
                mt_c[slots % 128, slots // 128, p, 3 * jj + 0] = m1[gb, u]
                mt_c[slots % 128, slots // 128, p, 3 * jj + 1] = m2[gb, u]
                mt_c[slots % 128, slots // 128, p, 3 * jj + 2] = (u == 0)
                scl_c[3 * jj + 0, p] = 1.0 / (hi1[gb, 0] - lo1[gb, 0])
                scl_c[3 * jj + 1, p] = 1.0 / (hi2[gb, 0] - lo2[gb, 0])
                scl_c[3 * jj + 2, p] = 1.0
                off += k
        # wrap idx for the Q7 cores: w[p_part, p, cblk] = idx[p, cblk*16 +
        # p_part%16], replicated over the 8 groups of 16 partitions
        w = idx_c.reshape(NP, 2 * S // 16, 16).transpose(2, 0, 1)
        gidx.append(np.ascontiguousarray(np.tile(w, (8, 1, 1))))
        mt.append(mt_c)
        scl.append(scl_c)
    return {"rows": rows, "pcnt": pcnt, "cq": cq, "cqmax": cqmax,
            "gidx": gidx, "mt": mt, "scl": scl}


def build_program(bpc=BPC, s=S, d=D, h1=H1, h2=H2, h3=H3, rep=1,
                  xbufs=3, pcnt=None, cqmax=None):
    """Emit the per-core Bass/Tile program. All 8 cores run it SPMD."""
    if pcnt is None:
        pcnt = _LAYOUT_CACHE["layout"]["pcnt"]
        cqmax = _LAYOUT_CACHE["layout"]["cqmax"]
    np_ = bpc // 2
    dh = d // 2            # moving free-dim per span matmul
    nd = d // 128          # d-chunks of 128
    nf = 3 * d // 128      # feature chunks of 128
    nh1 = h1 // 128
    cq = [(int(pcnt[p, 0] + pcnt[p, 1]) + 127) // 128 for p in range(np_)]
    assert max(cq) <= cqmax

    nc = bacc.Bacc("TRN2", target_bir_lowering=False, debug=False,
                   num_devices=N_CORES, num_swdge_queues=4)

    x_d = nc.dram_tensor("x", [bpc * s, d], F32R, kind="ExternalInput")
    gi_d = nc.dram_tensor("gidx", [128, np_, 2 * s // 16], I16,
                          kind="ExternalInput")
    mt_d = nc.dram_tensor("mt", [128, cqmax, np_, 6], F32R,
                          kind="ExternalInput")
    sc_d = nc.dram_tensor("scl", [6, np_], F32, kind="ExternalInput")
    w1_d = nc.dram_tensor("W1", [nf, 128, h1], F32, kind="ExternalInput")
    b1_d = nc.dram_tensor("b1", [1, h1], F32, kind="ExternalInput")
    w2_d = nc.dram_tensor("W2", [nh1, 128, h2], F32, kind="ExternalInput")
    b2_d = nc.dram_tensor("b2", [1, h2], F32, kind="ExternalInput")
    w3_d = nc.dram_tensor("W3", [h2, h3], F32, kind="ExternalInput")
    b3_d = nc.dram_tensor("b3", [1, h3], F32, kind="ExternalInput")
    out_d = nc.dram_tensor("out", [bpc, h3], F32, kind="ExternalOutput")

    with tile.TileContext(nc) as tc:
        with tc.tile_pool(name="const", bufs=1) as const:
            ident = const.tile([128, 128], F32)
            make_identity(nc, ident[:])

            w1 = const.tile([128, nf, h1], F32)
            nc.sync.dma_start(w1[:], w1_d.ap().rearrange("p k h -> k p h"))
            w2 = const.tile([128, nh1, h2], F32)
            nc.sync.dma_start(w2[:], w2_d.ap().rearrange("p k h -> k p h"))
            w3 = const.tile([h2, h3], F32)
            nc.sync.dma_start(w3[:], w3_d.ap()[:])
            b1r = const.tile([1, h1], F32)
            nc.sync.dma_start(b1r[:], b1_d.ap()[:])
            b2r = const.tile([1, h2], F32)
            nc.sync.dma_start(b2r[:], b2_d.ap()[:])
            b3r = const.tile([1, h3], F32)
            nc.sync.dma_start(b3r[:], b3_d.ap()[:])
            ones = const.tile([1, 128], F32)
            nc.vector.memset(ones[:], 1.0)

            for _rep in range(rep):
                gidx = const.tile([128, np_, 2 * s // 16], I16)
                nc.scalar.dma_start(gidx[:], gi_d.ap()[:])
                mt = const.tile([128, cqmax, np_, 6], F32R)
                nc.sync.dma_start(mt[:], mt_d.ap()[:])
                scl = const.tile([6, np_], F32)
                nc.sync.dma_start(scl[:], sc_d.ap()[:])

                # packT[d_p, dc, b, m]: transposed scaled span sums / cls.
                packT = const.tile([128, nd, bpc, 3], F32)

                # ---- main loop: gather span rows, accumulate sums on PE ----
                with tc.tile_pool(name="xp", bufs=xbufs) as xp, \
                     tc.tile_pool(name="stg", bufs=4) as stg, \
                     tc.tile_pool(name="sps0", bufs=2, space="PSUM") as sps0, \
                     tc.tile_pool(name="sps1", bufs=2, space="PSUM") as sps1, \
                     tc.tile_pool(name="ptp", bufs=2, space="PSUM") as ptp:
                    for p in range(np_):
                        xb = xp.tile([128, cqmax, d], F32R, tag="xb")
                        if _rep == 0 and p < xbufs:
                            # first rotation: ensure padding slots hold
                            # finite data (0 x garbage = NaN on the PE);
                            # memset can't encode f32r, so set as f32 bits
                            nc.vector.memset(xb[:].bitcast(F32), 0.0)
                        nc.gpsimd.dma_gather(
                            xb[:, 0:cq[p], :], x_d.ap()[bass.ts(p, 2 * s)],
                            gidx[:, p, :], 128 * cq[p],
                            int(pcnt[p, 0] + pcnt[p, 1]), d,
                            queue_num=p % 4)

                        sg = stg.tile([6, d], F32, tag="sg")
                        ps0 = sps0.tile([6, dh], F32, tag="ps0")
                        ps1 = sps1.tile([6, dh], F32, tag="ps1")
                        for c in range(cq[p]):
                            lhsT = mt[:, c, p, :]
                            nc.tensor.matmul(ps0[:], lhsT, xb[:, c, 0:dh],
                                             start=(c == 0),
                                             stop=(c == cq[p] - 1))
                            nc.tensor.matmul(ps1[:], lhsT, xb[:, c, dh:d],
                                             start=(c == 0),
                                             stop=(c == cq[p] - 1))
                        # evacuate + scale by 1/cnt (split DVE/ACT)
                        nc.vector.tensor_scalar(sg[:, 0:dh], ps0[:],
                                                scl[:, p:p + 1], None, OP.mult)
                        nc.scalar.mul(sg[:, dh:d], ps1[:], scl[:, p:p + 1])
                        # transpose [6, d] -> nd x [128, 6] into packT
                        for dc in range(nd):
                            pt = ptp.tile([128, 6], F32, tag="pt")
                            nc.tensor.transpose(pt[:], sg[:, bass.ts(dc, 128)],
                                                ident[0:6, 0:6])
                            for jj in range(2):
                                if (dc + jj) % 2 == 0:
                                    nc.vector.tensor_copy(
                                        packT[:, dc, 2 * p + jj, :],
                                        pt[:, 3 * jj:3 * jj + 3])
                                else:
                                    nc.scalar.copy(
                                        packT[:, dc, 2 * p + jj, :],
                                        pt[:, 3 * jj:3 * jj + 3])

                # ---- de-interleave features: featT[f_p, k, b] ----
                featT = const.tile([128, nf, bpc], F32)
                for m in range(3):
                    for dc in range(nd):
                        nc.vector.tensor_copy(featT[:, m * nd + dc, :],
                                              packT[:, dc, :, m])

                # ---- MLP + softmax over all bpc rows at once ----
                h1s = const.tile([bpc, h1], F32)
                h1T = const.tile([128, nh1, bpc], F32)
                h2s = const.tile([bpc, h2], F32)
                h2T = const.tile([h2, bpc], F32)
                probs = const.tile([bpc, h3], F32)
                mx = const.tile([bpc, 1], F32)
                ex = const.tile([bpc, h3], F32)
                sm = const.tile([bpc, 1], F32)
                rc = const.tile([bpc, 1], F32)

                with tc.tile_pool(name="mlpp", bufs=1, space="PSUM") as mp:
                    h1p = mp.tile([bpc, h1], F32, tag="h1p")
                    for k in range(nf):
                        nc.tensor.matmul(h1p[:], featT[:, k, :], w1[:, k, :],
                                         start=(k == 0), stop=False)
                    nc.tensor.matmul(h1p[:], ones[0:1, 0:bpc], b1r[:],
                                     start=False, stop=True)
                    nc.scalar.activation(h1s[:], h1p[:], AF.Relu)

                    for k in range(nh1):
                        tp1 = mp.tile([128, bpc], F32, tag="tp1")
                        nc.tensor.transpose(tp1[:], h1s[:, bass.ts(k, 128)],
                                            ident[0:bpc, 0:bpc])
                        nc.vector.tensor_copy(h1T[:, k, :], tp1[:])

                    h2p = mp.tile([bpc, h2], F32, tag="h2p")
                    for k in range(nh1):
                        nc.tensor.matmul(h2p[:], h1T[:, k, :], w2[:, k, :],
                                         start=(k == 0), stop=False)
                    nc.tensor.matmul(h2p[:], ones[0:1, 0:bpc], b2r[:],
                                     start=False, stop=True)
                    nc.scalar.activation(h2s[:], h2p[:], AF.Relu)

                    tp2 = mp.tile([h2, bpc], F32, tag="tp2")
                    nc.tensor.transpose(tp2[:], h2s[:], ident[0:bpc, 0:bpc])
                    nc.vector.tensor_copy(h2T[:], tp2[:])

                    h3p = mp.tile([bpc, h3], F32, tag="h3p")
                    nc.tensor.matmul(h3p[:], h2T[:], w3[:], start=True,
                                     stop=False)
                    nc.tensor.matmul(h3p[:], ones[0:1, 0:bpc], b3r[:],
                                     start=False, stop=True)

                    # softmax along the 4 logits
                    nc.vector.tensor_reduce(mx[:], h3p[:], mybir.AxisListType.X,
                                            OP.max, negate=True)
                    nc.scalar.activation(ex[:], h3p[:], AF.Exp, bias=mx[:],
                                         scale=1.0)
                    nc.vector.tensor_reduce(sm[:], ex[:], mybir.AxisListType.X,
                                            OP.add)
                    nc.vector.reciprocal(rc[:], sm[:])
                    nc.vector.tensor_scalar(probs[:], ex[:], rc[:], None,
                                            OP.mult)

                nc.sync.dma_start(out_d.ap()[:], probs[:])

    nc.compile()
    return nc


_LAYOUT_CACHE = {}
_NC_CACHE = {}


def _get_program(layout):
    key = tuple(int(c) for c in layout["pcnt"].ravel())
    if _NC_CACHE.get("key") != key:
        _NC_CACHE["nc"] = build_program(pcnt=layout["pcnt"],
                                        cqmax=layout["cqmax"])
        _NC_CACHE["key"] = key
    return _NC_CACHE["nc"]


def make_in_maps(inputs):
    x = np.ascontiguousarray(np.asarray(inputs["x"], dtype=np.float32))
    e1 = np.ascontiguousarray(np.asarray(inputs["e1_span"], dtype=np.int32))
    e2 = np.ascontiguousarray(np.asarray(inputs["e2_span"], dtype=np.int32))
    w1 = np.ascontiguousarray(
        np.asarray(inputs["W1"], dtype=np.float32).reshape(3 * D // 128, 128, H1))
    b1 = np.asarray(inputs["b1"], dtype=np.float32).reshape(1, H1)
    w2 = np.ascontiguousarray(
        np.asarray(inputs["W2"], dtype=np.float32).reshape(H1 // 128, 128, H2))
    b2 = np.asarray(inputs["b2"], dtype=np.float32).reshape(1, H2)
    w3 = np.ascontiguousarray(np.asarray(inputs["W3"], dtype=np.float32))
    b3 = np.asarray(inputs["b3"], dtype=np.float32).reshape(1, H3)

    layout = derive_layout(e1, e2)
    _LAYOUT_CACHE["layout"] = layout

    in_maps = []
    for c in range(N_CORES):
        rows = layout["rows"][c]
        in_maps.append({
            "x": np.ascontiguousarray(x[rows].reshape(BPC * S, D)),
            "gidx": layout["gidx"][c],
            "mt": layout["mt"][c],
            "scl": layout["scl"][c],
            "W1": w1, "b1": b1, "W2": w2, "b2": b2, "W3": w3, "b3": b3,
        })
    return in_maps


def kernel(**inputs) -> np.ndarray:
    in_maps = make_in_maps(inputs)
    layout = _LAYOUT_CACHE["layout"]
    nc = _get_program(layout)
    res = run_bass_kernel_spmd(nc, in_maps, core_ids=list(range(N_CORES)))
    out = np.empty((B_FULL, H3), dtype=np.float32)
    for c in range(N_CORES):
        out[layout["rows"][c]] = res.results[c]["out"]
    return out


# revision 8
# speedup vs baseline: 2.7613x; 1.5618x over previous
"""Trainium2 Bass kernel for BertTempRel-style span-mean + MLP + softmax.

Reference computation (per batch row b of x[B, S, D]):
    e1 = mean(x[b, lo1:hi1, :]),  e2 = mean(x[b, lo2:hi2, :]),  cls = x[b, 0, :]
    (hi = max(hi, lo+1) empty-span guard)
    feat = concat([e1, e2, cls])            # [3D]
    out = softmax(relu(relu(feat@W1+b1)@W2+b2)@W3+b3)

Strategy: pure data-parallel over 8 NeuronCores. Only ~54% of x's rows are
inside the two spans (plus CLS), so instead of streaming all of x, each
core issues one SWDGE dma_gather per PAIR of batch rows that reads exactly
the union-of-spans rows into packed SBUF slots. The host derives the row
lists from the tiny span tensors, sorts the 1024 batch rows by union size
and deals them round-robin across cores (so the 8 rows sharing a loop
position have near-equal counts), then pairs head and tail positions so
every gather moves a near-constant ~560 rows. The per-position counts are
baked into the program as immediates (SPMD-safe: register-valued DMA
operands fail on this deployment); shorter rows pad with dummy row-0
reads. The 0/1 span masks (3 per batch row, 6 per pair) are precomputed
host-side directly in the transposed [slot, mask] layout the TensorEngine
needs, so the device does no mask computation at all: per pair it runs
cq ~= 5 accumulating matmuls (stationary masks x moving gathered rows),
scales the [6, D] PSUM result by host-provided 1/count factors, transposes
batches of 12 rows back to feature-major layout, and finishes with the
128-row MLP + softmax as a handful of matmuls.
"""

import sys

if "/opt/trn_rl_repo" not in sys.path:
    sys.path.insert(0, "/opt/trn_rl_repo")

import numpy as np

from concourse import bacc, bass, mybir, tile
from concourse.bass_utils import run_bass_kernel_spmd
from concourse.masks import make_identity

F32 = mybir.dt.float32
F32R = mybir.dt.float32r
I16 = mybir.dt.int16
I32 = mybir.dt.int32
OP = mybir.AluOpType
AF = mybir.ActivationFunctionType

N_CORES = 8
B_FULL, S, D = 1024, 512, 768
H1, H2, H3 = 256, 64, 4
BPC = B_FULL // N_CORES  # batch rows per core (128)
NP = BPC // 2            # gather pairs per core (64)


def derive_layout(e1_span, e2_span):
    """Host-side layout: row lists, core/pair assignment, masks, scales.

    Returns dict with
      rows[c]    = [BPC] global batch row for each storage slot of core c
      pcnt       = [NP, 2] baked per-sub-row counts (shared by all cores)
      cq         = [NP] slot chunks per pair gather
      gidx[c]    = [128, NP, 2*S//16] int16 wrapped gather index lists
      mt[c]      = [128, cqmax, NP, 6] float32 transposed span masks
      scl[c]     = [6, NP] float32 1/count scales (e1, e2, cls=1) x 2 rows
    """
    e1 = np.asarray(e1_span, dtype=np.int64)
    e2 = np.asarray(e2_span, dtype=np.int64)
    pos = np.arange(S)[None, :]
    lo1, hi1 = e1[:, 0:1], np.maximum(e1[:, 1:2], e1[:, 0:1] + 1)
    lo2, hi2 = e2[:, 0:1], np.maximum(e2[:, 1:2], e2[:, 0:1] + 1)
    m1 = (pos >= lo1) & (pos < hi1)
    m2 = (pos >= lo2) & (pos < hi2)
    m = m1 | m2
    m[:, 0] = True  # CLS row
    n = m.sum(axis=1)

    order = np.argsort(-n, kind="stable")
    perm = order.reshape(BPC, N_CORES)          # position i, core c
    counts = n[perm[:, 0]].astype(np.int64)     # descending => max of group

    # head-tail pairing: pair p serves positions (p, BPC-1-p)
    pcnt = np.stack([counts[:NP], counts[BPC - 1 - np.arange(NP)]], axis=1)
    cq = (pcnt.sum(axis=1) + 127) // 128
    cqmax = int(cq.max())

    rows, gidx, mt, scl = [], [], [], []
    for c in range(N_CORES):
        rows_c = np.empty(BPC, dtype=np.int64)
        rows_c[0::2] = perm[:NP, c]
        rows_c[1::2] = perm[BPC - 1 - np.arange(NP), c]
        rows.append(rows_c)

        idx_c = np.zeros((NP, 2 * S), dtype=np.int16)
        mt_c = np.zeros((128, cqmax, NP, 6), dtype=np.float32)
        scl_c = np.empty((6, NP), dtype=np.float32)
        for p in range(NP):
            off = 0
            for jj in range(2):
                gb = rows_c[2 * p + jj]
                u = np.flatnonzero(m[gb])
                k = int(pcnt[p, jj])
                # slot s of this pair holds source row idx within the
                # pair's 2S-row window: sub-row jj lives at jj*S + row
                idx_c[p, off:off + len(u)] = jj * S + u
                # dummy padding reads row 0 of sub-row jj (mask 0)
                idx_c[p, off + len(u):off + k] = jj * S
                slots = off + np.arange(len(u))
                mt_c[slots % 128, slots // 128, p, 3 * jj + 0] = m1[gb, u]
                mt_c[slots % 128, slots // 128, p, 3 * jj + 1] = m2[gb, u]
                mt_c[slots % 128, slots // 128, p, 3 * jj + 2] = (u == 0)
                scl_c[3 * jj + 0, p] = 1.0 / (hi1[gb, 0] - lo1[gb, 0])
                scl_c[3 * jj + 1, p] = 1.0 / (hi2[gb, 0] - lo2[gb, 0])
                scl_c[3 * jj + 2, p] = 1.0
                off += k
        # wrap idx for the Q7 cores: w[p_part, p, cblk] = idx[p, cblk*16 +
        # p_part%16], replicated over the 8 groups of 16 partitions
        w = idx_c.reshape(NP, 2 * S // 16, 16).transpose(2, 0, 1)
        gidx.append(np.ascontiguousarray(np.tile(w, (8, 1, 1))))
        mt.append(mt_c)
        scl.append(scl_c)
    return {"rows": rows, "pcnt": pcnt, "cq": cq, "cqmax": cqmax,
            "gidx": gidx, "mt": mt, "scl": scl}


def build_program(bpc=BPC, s=S, d=D, h1=H1, h2=H2, h3=H3, rep=1,
                  xbufs=3, pcnt=None, cqmax=None, lite=False):
    """Emit the per-core Bass/Tile program. All 8 cores run it SPMD."""
    if pcnt is None:
        pcnt = _LAYOUT_CACHE["layout"]["pcnt"]
        cqmax = _LAYOUT_CACHE["layout"]["cqmax"]
    np_ = bpc // 2
    dh = d // 2            # moving free-dim per span matmul
    nd = d // 128          # d-chunks of 128
    nf = 3 * d // 128      # feature chunks of 128
    nh1 = h1 // 128
    cq = [(int(pcnt[p, 0] + pcnt[p, 1]) + 127) // 128 for p in range(np_)]
    assert max(cq) <= cqmax

    nc = bacc.Bacc("TRN2", target_bir_lowering=False, debug=False,
                   num_devices=N_CORES, num_swdge_queues=4)

    x_d = nc.dram_tensor("x", [bpc * s, d], F32R, kind="ExternalInput")
    gi_d = nc.dram_tensor("gidx", [128, np_, 2 * s // 16], I16,
                          kind="ExternalInput")
    mt_d = nc.dram_tensor("mt", [128, cqmax, np_, 6], F32R,
                          kind="ExternalInput")
    sc_d = nc.dram_tensor("scl", [6, np_], F32, kind="ExternalInput")
    w1_d = nc.dram_tensor("W1", [nf, 128, h1], F32, kind="ExternalInput")
    b1_d = nc.dram_tensor("b1", [1, h1], F32, kind="ExternalInput")
    w2_d = nc.dram_tensor("W2", [nh1, 128, h2], F32, kind="ExternalInput")
    b2_d = nc.dram_tensor("b2", [1, h2], F32, kind="ExternalInput")
    w3_d = nc.dram_tensor("W3", [h2, h3], F32, kind="ExternalInput")
    b3_d = nc.dram_tensor("b3", [1, h3], F32, kind="ExternalInput")
    out_d = nc.dram_tensor("out", [bpc, h3], F32, kind="ExternalOutput")

    with tile.TileContext(nc) as tc:
        with tc.tile_pool(name="const", bufs=1) as const:
            ident = const.tile([128, 128], F32)
            make_identity(nc, ident[:])

            w1 = const.tile([128, nf, h1], F32)
            nc.sync.dma_start(w1[:], w1_d.ap().rearrange("p k h -> k p h"))
            w2 = const.tile([128, nh1, h2], F32)
            nc.sync.dma_start(w2[:], w2_d.ap().rearrange("p k h -> k p h"))
            w3 = const.tile([h2, h3], F32)
            nc.sync.dma_start(w3[:], w3_d.ap()[:])
            b1r = const.tile([1, h1], F32)
            nc.sync.dma_start(b1r[:], b1_d.ap()[:])
            b2r = const.tile([1, h2], F32)
            nc.sync.dma_start(b2r[:], b2_d.ap()[:])
            b3r = const.tile([1, h3], F32)
            nc.sync.dma_start(b3r[:], b3_d.ap()[:])
            ones = const.tile([1, 128], F32)
            nc.vector.memset(ones[:], 1.0)

            for _rep in range(rep):
                gidx = const.tile([128, np_, 2 * s // 16], I16)
                nc.scalar.dma_start(gidx[:], gi_d.ap()[:])
                mt = const.tile([128, cqmax, np_, 6], F32R)
                nc.sync.dma_start(mt[:], mt_d.ap()[:])
                scl = const.tile([6, np_], F32)
                nc.sync.dma_start(scl[:], sc_d.ap()[:])

                # packT[d_p, dc, b, m]: transposed scaled span sums / cls.
                packT = const.tile([128, nd, bpc, 3], F32)

                # ---- main loop: gather span rows, accumulate sums on PE ----
                with tc.tile_pool(name="xp", bufs=xbufs) as xp, \
                     tc.tile_pool(name="stg", bufs=4) as stg, \
                     tc.tile_pool(name="sps0", bufs=2, space="PSUM") as sps0, \
                     tc.tile_pool(name="sps1", bufs=2, space="PSUM") as sps1, \
                     tc.tile_pool(name="ptp", bufs=2, space="PSUM") as ptp:
                    for p in range(np_):
                        xb = xp.tile([128, cqmax, d], F32R, tag="xb")
                        if _rep == 0 and p < xbufs:
                            # first rotation: ensure padding slots hold
                            # finite data (0 x garbage = NaN on the PE);
                            # memset can't encode f32r, so set as f32 bits
                            nc.vector.memset(xb[:].bitcast(F32), 0.0)
                        nc.gpsimd.dma_gather(
                            xb[:, 0:cq[p], :], x_d.ap()[bass.ts(p, 2 * s)],
                            gidx[:, p, :], 128 * cq[p],
                            int(pcnt[p, 0] + pcnt[p, 1]), d,
                            queue_num=p % 4)

                        sg = stg.tile([6, d], F32, tag="sg")
                        ps0 = sps0.tile([6, dh], F32, tag="ps0")
                        ps1 = sps1.tile([6, dh], F32, tag="ps1")
                        for c in range(cq[p]):
                            lhsT = mt[:, c, p, :]
                            nc.tensor.matmul(ps0[:], lhsT, xb[:, c, 0:dh],
                                             start=(c == 0),
                                             stop=(c == cq[p] - 1))
                            nc.tensor.matmul(ps1[:], lhsT, xb[:, c, dh:d],
                                             start=(c == 0),
                                             stop=(c == cq[p] - 1))
                        # evacuate + scale by 1/cnt (split DVE/ACT)
                        nc.vector.tensor_scalar(sg[:, 0:dh], ps0[:],
                                                scl[:, p:p + 1], None, OP.mult)
                        nc.scalar.mul(sg[:, dh:d], ps1[:], scl[:, p:p + 1])
                        if lite:
                            continue
                        # transpose [6, d] -> nd x [128, 6] into packT
                        for dc in range(nd):
                            pt = ptp.tile([128, 6], F32, tag="pt")
                            nc.tensor.transpose(pt[:], sg[:, bass.ts(dc, 128)],
                                                ident[0:6, 0:6])
                            for jj in range(2):
                                if (dc + jj) % 2 == 0:
                                    nc.vector.tensor_copy(
                                        packT[:, dc, 2 * p + jj, :],
                                        pt[:, 3 * jj:3 * jj + 3])
                                else:
                                    nc.scalar.copy(
                                        packT[:, dc, 2 * p + jj, :],
                                        pt[:, 3 * jj:3 * jj + 3])

                if lite:
                    # probe build: skip transposes/MLP, emit a dummy output
                    probs_l = const.tile([bpc, h3], F32)
                    nc.vector.memset(probs_l[:], 0.5)
                    nc.sync.dma_start(out_d.ap()[:], probs_l[:])
                    continue

                # ---- de-interleave features: featT[f_p, k, b] ----
                featT = const.tile([128, nf, bpc], F32)
                for m in range(3):
                    for dc in range(nd):
                        nc.vector.tensor_copy(featT[:, m * nd + dc, :],
                                              packT[:, dc, :, m])

                # ---- MLP + softmax over all bpc rows at once ----
                h1s = const.tile([bpc, h1], F32)
                h1T = const.tile([128, nh1, bpc], F32)
                h2s = const.tile([bpc, h2], F32)
                h2T = const.tile([h2, bpc], F32)
                probs = const.tile([bpc, h3], F32)
                mx = const.tile([bpc, 1], F32)
                ex = const.tile([bpc, h3], F32)
                sm = const.tile([bpc, 1], F32)
                rc = const.tile([bpc, 1], F32)

                with tc.tile_pool(name="mlpp", bufs=1, space="PSUM") as mp:
                    h1p = mp.tile([bpc, h1], F32, tag="h1p")
                    for k in range(nf):
                        nc.tensor.matmul(h1p[:], featT[:, k, :], w1[:, k, :],
                                         start=(k == 0), stop=False)
                    nc.tensor.matmul(h1p[:], ones[0:1, 0:bpc], b1r[:],
                                     start=False, stop=True)
                    nc.scalar.activation(h1s[:], h1p[:], AF.Relu)

                    for k in range(nh1):
                        tp1 = mp.tile([128, bpc], F32, tag="tp1")
                        nc.tensor.transpose(tp1[:], h1s[:, bass.ts(k, 128)],
                                            ident[0:bpc, 0:bpc])
                        nc.vector.tensor_copy(h1T[:, k, :], tp1[:])

                    h2p = mp.tile([bpc, h2], F32, tag="h2p")
                    for k in range(nh1):
                        nc.tensor.matmul(h2p[:], h1T[:, k, :], w2[:, k, :],
                                         start=(k == 0), stop=False)
                    nc.tensor.matmul(h2p[:], ones[0:1, 0:bpc], b2r[:],
                                     start=False, stop=True)
                    nc.scalar.activation(h2s[:], h2p[:], AF.Relu)

                    tp2 = mp.tile([h2, bpc], F32, tag="tp2")
                    nc.tensor.transpose(tp2[:], h2s[:], ident[0:bpc, 0:bpc])
                    nc.vector.tensor_copy(h2T[:], tp2[:])

                    h3p = mp.tile([bpc, h3], F32, tag="h3p")
                    nc.tensor.matmul(h3p[:], h2T[:], w3[:], start=True,
                                     stop=False)
                    nc.tensor.matmul(h3p[:], ones[0:1, 0:bpc], b3r[:],
                                     start=False, stop=True)

                    # softmax along the 4 logits
                    nc.vector.tensor_reduce(mx[:], h3p[:], mybir.AxisListType.X,
                                            OP.max, negate=True)
                    nc.scalar.activation(ex[:], h3p[:], AF.Exp, bias=mx[:],
                                         scale=1.0)
                    nc.vector.tensor_reduce(sm[:], ex[:], mybir.AxisListType.X,
                                            OP.add)
                    nc.vector.reciprocal(rc[:], sm[:])
                    nc.vector.tensor_scalar(probs[:], ex[:], rc[:], None,
                                            OP.mult)

                nc.sync.dma_start(out_d.ap()[:], probs[:])

    nc.compile()
    return nc


_LAYOUT_CACHE = {}
_NC_CACHE = {}


def _get_program(layout):
    key = tuple(int(c) for c in layout["pcnt"].ravel())
    if _NC_CACHE.get("key") != key:
        _NC_CACHE["nc"] = build_program(pcnt=layout["pcnt"],
                                        cqmax=layout["cqmax"])
        _NC_CACHE["key"] = key
    return _NC_CACHE["nc"]


def make_in_maps(inputs):
    x = np.ascontiguousarray(np.asarray(inputs["x"], dtype=np.float32))
    e1 = np.ascontiguousarray(np.asarray(inputs["e1_span"], dtype=np.int32))
    e2 = np.ascontiguousarray(np.asarray(inputs["e2_span"], dtype=np.int32))
    w1 = np.ascontiguousarray(
        np.asarray(inputs["W1"], dtype=np.float32).reshape(3 * D // 128, 128, H1))
    b1 = np.asarray(inputs["b1"], dtype=np.float32).reshape(1, H1)
    w2 = np.ascontiguousarray(
        np.asarray(inputs["W2"], dtype=np.float32).reshape(H1 // 128, 128, H2))
    b2 = np.asarray(inputs["b2"], dtype=np.float32).reshape(1, H2)
    w3 = np.ascontiguousarray(np.asarray(inputs["W3"], dtype=np.float32))
    b3 = np.asarray(inputs["b3"], dtype=np.float32).reshape(1, H3)

    layout = derive_layout(e1, e2)
    _LAYOUT_CACHE["layout"] = layout

    in_maps = []
    for c in range(N_CORES):
        rows = layout["rows"][c]
        in_maps.append({
            "x": np.ascontiguousarray(x[rows].reshape(BPC * S, D)),
            "gidx": layout["gidx"][c],
            "mt": layout["mt"][c],
            "scl": layout["scl"][c],
            "W1": w1, "b1": b1, "W2": w2, "b2": b2, "W3": w3, "b3": b3,
        })
    return in_maps


def kernel(**inputs) -> np.ndarray:
    in_maps = make_in_maps(inputs)
    layout = _LAYOUT_CACHE["layout"]
    nc = _get_program(layout)
    res = run_bass_kernel_spmd(nc, in_maps, core_ids=list(range(N_CORES)))
    out = np.empty((B_FULL, H3), dtype=np.float32)
    for c in range(N_CORES):
        out[layout["rows"][c]] = res.results[c]["out"]
    return out
